# revision 12
# baseline (speedup 1.0000x reference)
"""Self-contained Trainium2 Bass kernel for the CharRNN problem:
2-layer LSTM (B=32, T=256, H=256) + V=32000 softmax cross-entropy mean loss.

Strategy (8 NeuronCores, SPMD):
  * LSTM recurrence replicated on every core (latency-bound); softmax
    sharded over vocab (VS=4000/core); host combines partial sums.
  * Per-step gates are computed with COLUMN-TILED matmuls
    (tile_position=(0,32j)): partition strip j (rows 32j:32j+32) holds
    batch rows for HIDDEN QUARTER j, with the strip's 256 psum columns
    = [i|o|f|jnew] x 64 units. The 4 strip matmuls stream concurrently
    on the PE sub-arrays, and the gate nonlinearity becomes ONE
    128-partition Tanh ACT instr (sigmoid = 0.5*tanh(x/2)+0.5 with the
    0.5 pre-scaled into W, forget bias injected via a K=1 ones-row
    matmul in the accumulation group).
  * All elementwise state math is [128, 64] (batch x quarter
    interleaved); h is transposed back to hidden-major k-tiles with 4
    tiny PE transposes (tile_position row/col placement) + 1 DVE copy
    instead of 1.2us DMA transposes.
  * Softmax: logits matmuls into 2-bank psum pairs, exp via wide ACT
    instrs with accum_out; per-row target logit via gpsimd ap_gather
    (int16-pair view of sw) + multiply + ones-matmul reduce.
  * Host combines: loss_r = log(sum_cores se_r) - tgt_logit_r.
"""
import os
import numpy as np
import ml_dtypes
import concourse.bass as bass
import concourse.mybir as mybir
import concourse.tile as tile
from concourse import bacc
from concourse.masks import make_identity
from concourse.bass_utils import run_bass_kernel_spmd

F32 = mybir.dt.float32
BF16 = mybir.dt.bfloat16
I32 = mybir.dt.int32
I16 = mybir.dt.int16
AF = mybir.ActivationFunctionType
ALU = mybir.AluOpType

B, T, H, V, NCORES = 32, 256, 256, 32000, 8


def build_charrnn(T=256, V=32000, n_cores=8, has_swb=False, num_devices=8):
    B, H = 32, 256
    G4 = 4 * H
    VS = V // n_cores
    BT = B * T
    RT = BT // 128                  # 128-row tiles (4 steps each)
    assert T % 4 == 0 and BT % 128 == 0

    CH = 500                        # logits chunk (<=512 = one psum bank)
    NCHUNK = VS // CH               # 8 chunks per tile
    assert VS % CH == 0 and NCHUNK % 2 == 0
    NEXP = NCHUNK // 2              # exp instrs per tile (2 chunks each)

    nc = bacc.Bacc("TRN2", target_bir_lowering=False, debug=False,
                   num_devices=num_devices)

    # ---------------- DRAM I/O ----------------
    ids_d = nc.dram_tensor("ids", (RT, 128, 1), I32, kind="ExternalInput")
    emb_d = nc.dram_tensor("emb", (V, H), BF16, kind="ExternalInput")
    w1_d = nc.dram_tensor("w1", (4, 128, G4), BF16, kind="ExternalInput")
    w2_d = nc.dram_tensor("w2", (4, 128, G4), BF16, kind="ExternalInput")
    br1_d = nc.dram_tensor("brow1", (1, G4), BF16, kind="ExternalInput")
    br2_d = nc.dram_tensor("brow2", (1, G4), BF16, kind="ExternalInput")
    sw_d = nc.dram_tensor("sw", (2, 128, VS), BF16, kind="ExternalInput")
    swp_d = nc.dram_tensor("swp", (2, 128, VS, 2), I16, kind="ExternalInput")
    tgi_d = nc.dram_tensor("tgi", (RT, 128, 8), I16, kind="ExternalInput")
    if has_swb:
        swb_d = nc.dram_tensor("swbp", (128, VS), F32, kind="ExternalInput")
    se_d = nc.dram_tensor("se_out", (128, RT * NEXP), F32,
                          kind="ExternalOutput")
    tg_d = nc.dram_tensor("tg_out", (1, BT), F32, kind="ExternalOutput")

    with tile.TileContext(nc) as tc:
        with tc.tile_pool(name="persist", bufs=1) as pp:
            # ---- persistent SBUF ----
            w1_sb = pp.tile([128, 4, G4], BF16, tag="w1")
            w2_sb = pp.tile([128, 4, G4], BF16, tag="w2")
            nc.sync.dma_start(w1_sb[:], w1_d[:].rearrange("k p c -> p k c"))
            nc.sync.dma_start(w2_sb[:], w2_d[:].rearrange("k p c -> p k c"))
            br1 = pp.tile([1, G4], BF16, tag="br1")
            br2 = pp.tile([1, G4], BF16, tag="br2")
            nc.sync.dma_start(br1[:], br1_d[:])
            nc.sync.dma_start(br2[:], br2_d[:])
            sw_sb = pp.tile([128, 2, VS], BF16, tag="sw")
            nc.sync.dma_start(sw_sb[:], sw_d[:].rearrange("k p c -> p k c"))
            swp_sb = pp.tile([128, 2, VS, 2], I16, tag="swp")
            nc.sync.dma_start(swp_sb[:],
                              swp_d[:].rearrange("k p c d -> p k c d"))
            if has_swb:
                swb_sb = pp.tile([128, VS], F32, tag="swb")
                nc.sync.dma_start(swb_sb[:], swb_d[:])

            xsT = pp.tile([128, 2, BT], BF16, tag="xsT")
            hsT = pp.tile([128, 2, BT], BF16, tag="hsT")

            ones1 = pp.tile([1, 32], BF16, tag="ones1")
            nc.gpsimd.memset(ones1[:], 1.0)
            ones4 = pp.tile([1, 128], BF16, tag="ones4")
            nc.gpsimd.memset(ones4[:], 1.0)
            onesc = pp.tile([128, 1], BF16, tag="onesc")
            nc.gpsimd.memset(onesc[:], 1.0)
            ident = pp.tile([128, 128], BF16, tag="ident")
            make_identity(nc, ident[:])

            c1 = pp.tile([128, 64], F32, tag="c1")
            c2 = pp.tile([128, 64], F32, tag="c2")
            nc.gpsimd.memset(c1[:], 0.0)
            nc.gpsimd.memset(c2[:], 0.0)
            junk = pp.tile([128, 1], F32, tag="junk")

            h1T = pp.tile([128, 2, 32], BF16, tag="h1T")

            se_sb = pp.tile([128, RT * NEXP], F32, tag="se")
            tg_sb = pp.tile([1, BT], F32, tag="tg")
            # accum_out adds into existing SBUF content on HW — zero it
            nc.gpsimd.memset(se_sb[:], 0.0)

            with (
                tc.tile_pool(name="stage", bufs=3) as stp,
                tc.tile_pool(name="gwork", bufs=2) as gw,
                tc.tile_pool(name="lwork", bufs=2) as lw,
                tc.tile_pool(name="z1p", bufs=1, space="PSUM") as z1p,
                tc.tile_pool(name="z2p", bufs=1, space="PSUM") as z2p,
                tc.tile_pool(name="lgp", bufs=2, space="PSUM") as lgp,
                tc.tile_pool(name="htp", bufs=1, space="PSUM") as htp,
                tc.tile_pool(name="ptp", bufs=1, space="PSUM") as ptp,
                tc.tile_pool(name="ew", bufs=3) as ew,
            ):
                # ---- embedding gather (time-major) + transpose to slabs ----
                for rt in range(RT):
                    ids_sb = stp.tile([128, 1], I32, tag="ids")
                    nc.gpsimd.dma_start(ids_sb[:], ids_d.ap()[rt])
                    xrow = stp.tile([128, H], BF16, tag="xrow")
                    nc.gpsimd.indirect_dma_start(
                        out=xrow[:], out_offset=None,
                        in_=emb_d[:],
                        in_offset=bass.IndirectOffsetOnAxis(
                            ap=ids_sb[:, :1], axis=0),
                    )
                    cs = 128 * rt
                    nc.sync.dma_start_transpose(
                        xsT[:, 0, cs:cs + 128], xrow[:, 0:128])
                    nc.sync.dma_start_transpose(
                        xsT[:, 1, cs:cs + 128], xrow[:, 128:256])

                def emit_pair(rt, s):
                    """Logits matmuls for chunk-pair s of row-tile rt.
                    Returns a closure that emits the exp (run ~1 step
                    later so the ACT never waits on these matmuls)."""
                    cs = 128 * rt
                    lg = lgp.tile([128, 2, 512], F32, tag="lg")
                    for k in range(2):
                        nc.tensor.ldweights(hsT[:, k, cs:cs + 128])
                        for half in range(2):
                            ch = s * 2 + half
                            mm = nc.tensor.matmul(
                                lg[:, half, 0:CH],
                                hsT[:, k, cs:cs + 128],
                                sw_sb[:, k, ch * CH:ch * CH + CH],
                                start=(k == 0), stop=(k == 1),
                            )
                            mm.ldweights = False

                    def do_exp():
                        if has_swb:
                            for half in range(2):
                                ch = s * 2 + half
                                nc.vector.tensor_tensor(
                                    out=lg[:, half, 0:CH],
                                    in0=lg[:, half, 0:CH],
                                    in1=swb_sb[:, ch * CH:ch * CH + CH],
                                    op=ALU.add)
                        ebuf = ew.tile([128, 2, CH], BF16, tag="ebuf")
                        col = rt * NEXP + s
                        nc.scalar.activation(
                            ebuf[:], lg[:, :, 0:CH], AF.Exp,
                            accum_out=se_sb[:, col:col + 1])
                    return do_exp

                def emit_tgt(rt):
                    # target logit for row-tile rt's 128 rows
                    cs = 128 * rt
                    tgi_sb = ew.tile([128, 8], I16, tag="tgi")
                    nc.gpsimd.dma_start(tgi_sb[:], tgi_d.ap()[rt])
                    pst = ptp.tile([1, 128], F32, tag="pst")
                    for k in range(2):
                        swg = ew.tile([128, 128, 2], I16, tag="swg")
                        nc.gpsimd.ap_gather(
                            swg[:], swp_sb[:, k], tgi_sb[:],
                            channels=128, num_elems=VS, d=2, num_idxs=128,
                        )
                        mulk = ew.tile([128, 128], BF16, tag="mulk")
                        nc.vector.tensor_tensor(
                            out=mulk[:],
                            in0=swg[:].bitcast(BF16)[:, :, 0],
                            in1=hsT[:, k, cs:cs + 128],
                            op=ALU.mult)
                        nc.tensor.matmul(pst[:], onesc[:, 0:1], mulk[:],
                                         start=(k == 0), stop=(k == 1))
                    nc.scalar.copy(tg_sb[0:1, cs:cs + 128], pst[:])

                def lstm_layer(zpool, w_sb, brow, c_sb, xks, hks, hTdst):
                    """One layer step. xks/hks: list of (bcast ldw AP,
                    plain lhsT AP, k index). hTdst: psum AP [128, 64]
                    for the h transposes. Returns h_int."""
                    z = zpool.tile([128, 256], F32, tag="z")
                    nk = len(xks) + len(hks)
                    nc.tensor.ldweights(ones4[0:1, :])
                    for j in range(4):
                        mm = nc.tensor.matmul(
                            z[32 * j:32 * j + 32, :], ones1[0:1, :],
                            brow[0:1, 256 * j:256 * j + 256],
                            start=True, stop=False,
                            tile_position=(0, 32 * j), skip_group_check=True)
                        mm.ldweights = False
                    for i, (ltb, lt, k) in enumerate(xks + hks):
                        last = (i == nk - 1)
                        nc.tensor.ldweights(ltb)
                        for j in range(4):
                            mm = nc.tensor.matmul(
                                z[32 * j:32 * j + 32, :], lt,
                                w_sb[:, k, 256 * j:256 * j + 256],
                                start=False, stop=last,
                                tile_position=(0, 32 * j),
                                skip_group_check=True)
                            mm.ldweights = False
                    # one Tanh for all gates: cols [i|o|f|j] x 64
                    g = gw.tile([128, 256], BF16, tag="g")
                    nc.scalar.activation(g[:], z[:], AF.Tanh)
                    # c = sig(f+1)*c + sig(i)*tanh(j); h = tanh(c)*sig(o)
                    u = lw.tile([128, 64], BF16, tag="u")
                    nc.vector.affine_mul_reduce(
                        u[:], junk[:], g[:, 0:64], g[:, 192:256], 0.5, 0.5)
                    v = lw.tile([128, 64], F32, tag="v")
                    nc.vector.affine_mul_reduce(
                        v[:], junk[:], g[:, 128:192], c_sb[:], 0.5, 0.5)
                    nc.vector.tensor_tensor(out=c_sb[:], in0=u[:], in1=v[:],
                                            op=ALU.add)
                    tc_t = lw.tile([128, 64], BF16, tag="tc")
                    nc.scalar.activation(tc_t[:], c_sb[:], AF.Tanh)
                    hrow = lw.tile([128, 1, 64], BF16, tag="hrow")
                    nc.vector.affine_mul_reduce(
                        hrow[:, 0, :], junk[:], g[:, 64:128], tc_t[:],
                        0.5, 0.5)
                    # transpose to hidden-major k-tiles:
                    # quarter j -> [64, 32] at (partition 64*(j%2), col 32*(j//2))
                    nc.tensor.ldweights(
                        hrow[:, 0:1, :].broadcast_to([128, 2, 64]),
                        is_transpose=True)
                    for j in range(4):
                        pj = 64 * (j % 2)
                        cj = 32 * (j // 2)
                        mm = nc.tensor.transpose(
                            hTdst[pj:pj + 64, cj:cj + 32],
                            hrow[32 * j:32 * j + 32, 0, :],
                            ident[32 * j:32 * j + 32, 32 * j:32 * j + 32],
                            tile_position=(32 * j, pj),
                        )
                        mm.ldweights = False
                    return hrow

                # ---- LSTM over T steps ----
                exp_q = []
                for t in range(T):
                    ts0 = 32 * t
                    hpair = htp.tile([128, 128], BF16, tag="hpair")
                    # layer 1: x k-tiles (k=0,1) + h1 k-tiles (k=2,3)
                    xks = [(xsT[:, k:k + 1, ts0:ts0 + 32].broadcast_to(
                        [128, 4, 32]), xsT[:, k, ts0:ts0 + 32], k)
                        for k in range(2)]
                    hks = [] if t == 0 else \
                        [(h1T[:, k:k + 1, :].broadcast_to([128, 4, 32]),
                          h1T[:, k, :], 2 + k) for k in range(2)]
                    lstm_layer(z1p, w1_sb, br1, c1, xks, hks, hpair[:, 0:64])
                    nc.vector.tensor_copy(h1T[:], hpair[:, 0:64])

                    # layer 2: h2 k-tiles (prev step, k=2,3) + h1 (k=0,1)
                    hks2 = [] if t == 0 else \
                        [(hsT[:, k:k + 1, ts0 - 32:ts0].broadcast_to(
                            [128, 4, 32]), hsT[:, k, ts0 - 32:ts0], 2 + k)
                         for k in range(2)]
                    xks2 = [(h1T[:, k:k + 1, :].broadcast_to([128, 4, 32]),
                             h1T[:, k, :], k) for k in range(2)]
                    lstm_layer(z2p, w2_sb, br2, c2, hks2, xks2,
                               hpair[:, 64:128])
                    nc.vector.tensor_copy(hsT[:, 0:2, ts0:ts0 + 32],
                                          hpair[:, 64:128])

                    # softmax: one chunk-pair of tile rt per step, spread
                    # over steps 4rt+3 .. 4rt+6; exp runs one step later
                    if t >= 3:
                        rt, s = divmod(t - 3, 4)
                        if s == 0:
                            emit_tgt(rt)
                        exp_q.append(emit_pair(rt, s))
                    while len(exp_q) > 1:
                        exp_q.pop(0)()

                for tt in range(T, T + 3):
                    rt, s = divmod(tt - 3, 4)
                    if s == 0:
                        emit_tgt(rt)
                    exp_q.append(emit_pair(rt, s))
                while exp_q:
                    exp_q.pop(0)()

            nc.sync.dma_start(se_d[:], se_sb[:])
            nc.sync.dma_start(tg_d[:], tg_sb[:])

    nc.compile()
    meta = dict(T=T, V=V, n_cores=n_cores, B=B, H=H, VS=VS, BT=BT, RT=RT,
                CH=CH, NCHUNK=NCHUNK, NEXP=NEXP)
    return nc, meta


# ---------------- host-side prep / combine ----------------

def prep_inputs(meta, input_data, targets, embedding, W1, b1, W2, b2,
                softmax_w, softmax_b):
    """Build the per-core input maps (numpy)."""
    B, T, V = meta["B"], meta["T"], meta["V"]
    VS, RT, n_cores = meta["VS"], meta["RT"], meta["n_cores"]
    H = meta["H"]
    G4 = 4 * H

    ids_tm = np.ascontiguousarray(
        np.asarray(input_data, np.int64).T).reshape(-1)
    tgt_tm = np.ascontiguousarray(
        np.asarray(targets, np.int64).T).reshape(-1)
    ids_in = ids_tm.astype(np.int32).reshape(RT, 128, 1)

    # W column permutation: new col = 256*jq + 64*g + u  <-  tf col
    # tfblock(g)*256 + 64*jq + u, g order [i,o,f,jnew] -> tf [i,j,f,o]
    tfblock = [0, 3, 2, 1]
    jq = np.arange(G4) // 256
    g = (np.arange(G4) % 256) // 64
    u = np.arange(G4) % 64
    perm = np.array(tfblock)[g] * 256 + 64 * jq + u
    scale = np.where(g < 3, 0.5, 1.0).astype(np.float32)  # i,o,f sigmoid

    def prep_w(W):
        Wp = (np.asarray(W, np.float32)[:, perm] * scale[None, :]).astype(
            ml_dtypes.bfloat16)
        return np.ascontiguousarray(Wp.reshape(4, 128, G4))

    def prep_b(b):
        bp = np.asarray(b, np.float32)[perm] * scale
        bp = bp + np.where(g == 2, 0.5, 0.0)      # forget bias (scaled)
        return np.ascontiguousarray(
            bp.astype(ml_dtypes.bfloat16).reshape(1, G4))

    w1_in = prep_w(W1)
    w2_in = prep_w(W2)
    br1 = prep_b(b1)
    br2 = prep_b(b2)

    emb_in = np.ascontiguousarray(
        np.asarray(embedding, np.float32).astype(ml_dtypes.bfloat16))

    sw = np.asarray(softmax_w, np.float32)                  # [H, V]
    swb = np.asarray(softmax_b, np.float32)
    has_swb = bool(np.any(swb))

    # vectorized ap_gather index layout: idx i lives at partition i%16,
    # column i//16, replicated per 16-partition group
    rtA = (np.arange(RT) * 128)[:, None, None]
    pA = (np.arange(128) % 16)[None, :, None]
    qA = (np.arange(8) * 16)[None, None, :]
    gat = rtA + qA + pA                                     # [RT, 128, 8]

    maps, masks = [], []
    for c in range(n_cores):
        shard = sw[:, c * VS:(c + 1) * VS].astype(ml_dtypes.bfloat16)
        sw_in = np.ascontiguousarray(shard.reshape(2, 128, VS))
        swi = sw_in.view(np.int16)
        swp_in = np.ascontiguousarray(
            np.stack([swi, swi], axis=-1))                  # [2,128,VS,2]

        tl = tgt_tm - c * VS
        inr = (tl >= 0) & (tl < VS)
        tlc = np.where(inr, tl, 0).astype(np.int16)
        tgi = tlc[gat]                                      # [RT, 128, 8]
        m = dict(ids=ids_in, emb=emb_in,
                 w1=w1_in, w2=w2_in, brow1=br1, brow2=br2,
                 sw=sw_in, swp=swp_in, tgi=tgi)
        if has_swb:
            m["swbp"] = np.ascontiguousarray(
                np.tile(swb[c * VS:(c + 1) * VS].reshape(1, VS), (128, 1)))
        maps.append(m)
        masks.append(inr.astype(np.float32))
    return maps, masks, ids_tm, tgt_tm, has_swb


def combine_outputs(meta, results, masks, tgt_tm, softmax_b):
    """results: list of per-core dicts with se_out [128, RT*NEXP] and
    tg_out [1, BT]. Returns the scalar cost (np.float32)."""
    B, T, BT = meta["B"], meta["T"], meta["BT"]
    RT, NEXP = meta["RT"], meta["NEXP"]
    se_all = np.zeros(BT, np.float64)
    tg_all = np.zeros(BT, np.float64)
    for c, r in enumerate(results):
        se = np.asarray(r["se_out"], np.float64)  # [128, RT*NEXP]
        se = se.reshape(128, RT, NEXP).sum(-1)    # [128, RT]
        se_all += se.T.reshape(-1)                # row r = rt*128 + p
        tg_all += np.asarray(r["tg_out"], np.float64)[0] * masks[c]
    tg_all += np.asarray(softmax_b, np.float64)[tgt_tm]
    loss = np.log(se_all) - tg_all
    return np.float32(loss.sum() / B / T)


# ---------------- public entry point ----------------

_CACHE = {}
last_exec_time_ns = None
last_trace_path = None


def _get_built(T_, has_swb):
    key = (T_, has_swb)
    if key not in _CACHE:
        _CACHE[key] = build_charrnn(T=T_, V=V, n_cores=NCORES,
                                    has_swb=has_swb, num_devices=NCORES)
    return _CACHE[key]


def kernel(input_data, targets, embedding, W1, b1, W2, b2,
           softmax_w, softmax_b, _trace=False):
    global last_exec_time_ns, last_trace_path
    T_ = int(np.asarray(input_data).shape[1])
    has_swb = bool(np.any(np.asarray(softmax_b)))
    nc, meta = _get_built(T_, has_swb)
    maps, masks, ids_tm, tgt_tm, _ = prep_inputs(
        meta, input_data, targets, embedding, W1, b1, W2, b2,
        softmax_w, softmax_b)
    res = run_bass_kernel_spmd(nc, maps, core_ids=list(range(NCORES)),
                               trace=_trace)
    last_exec_time_ns = res.exec_time_ns
    if res.instructions_and_trace is not None:
        last_trace_path = res.instructions_and_trace[1]
    cost = combine_outputs(meta, res.results, masks, tgt_tm, softmax_b)
    return np.asarray(cost, np.float32)


# revision 19
# speedup vs baseline: 1.0755x; 1.0755x over previous
"""Self-contained Trainium2 Bass kernel for the CharRNN problem:
2-layer LSTM (B=32, T=256, H=256) + V=32000 softmax cross-entropy mean loss.

Strategy (8 NeuronCores, SPMD):
  * LSTM recurrence replicated on every core (latency-bound); softmax
    sharded over vocab (VS=4000/core); host combines partial sums.
  * Per-step gates are computed with COLUMN-TILED matmuls
    (tile_position=(0,32j)): partition strip j (rows 32j:32j+32) holds
    batch rows for HIDDEN QUARTER j, with the strip's 256 psum columns
    = [i|o|f|jnew] x 64 units. The 4 strip matmuls stream concurrently
    on the PE sub-arrays, and the gate nonlinearity becomes ONE
    128-partition Tanh ACT instr (sigmoid = 0.5*tanh(x/2)+0.5 with the
    0.5 pre-scaled into W, forget bias injected via a K=1 ones-row
    matmul in the accumulation group).
  * All elementwise state math is [128, 64] (batch x quarter
    interleaved); h is transposed back to hidden-major k-tiles with 4
    tiny PE transposes (tile_position row/col placement) + 1 DVE copy
    instead of 1.2us DMA transposes.
  * Softmax: logits matmuls into 2-bank psum pairs, exp via wide ACT
    instrs with accum_out; per-row target logit via gpsimd ap_gather
    (int16-pair view of sw) + multiply + ones-matmul reduce.
  * Host combines: loss_r = log(sum_cores se_r) - tgt_logit_r.
"""
import os
import numpy as np
import ml_dtypes
import concourse.bass as bass
import concourse.mybir as mybir
import concourse.tile as tile
from concourse import bacc
from concourse.masks import make_identity
from concourse.bass_utils import run_bass_kernel_spmd

F32 = mybir.dt.float32
BF16 = mybir.dt.bfloat16
I32 = mybir.dt.int32
I16 = mybir.dt.int16
AF = mybir.ActivationFunctionType
ALU = mybir.AluOpType

B, T, H, V, NCORES = 32, 256, 256, 32000, 8


def build_charrnn(T=256, V=32000, n_cores=8, has_swb=False, has_bias=False,
                  num_devices=8):
    B, H = 32, 256
    G4 = 4 * H
    VS = V // n_cores
    BT = B * T
    RT = BT // 128                  # 128-row tiles (4 steps each)
    assert T % 4 == 0 and BT % 128 == 0

    CH = 500                        # logits chunk (<=512 = one psum bank)
    NCHUNK = VS // CH               # 8 chunks per tile
    assert VS % CH == 0 and NCHUNK % 2 == 0
    NEXP = NCHUNK // 2              # exp instrs per tile (2 chunks each)

    nc = bacc.Bacc("TRN2", target_bir_lowering=False, debug=False,
                   num_devices=num_devices)

    # ---------------- DRAM I/O ----------------
    ids_d = nc.dram_tensor("ids", (RT, 128, 1), I32, kind="ExternalInput")
    emb_d = nc.dram_tensor("emb", (V, H), BF16, kind="ExternalInput")
    w1_d = nc.dram_tensor("w1", (4, 128, G4), BF16, kind="ExternalInput")
    w2_d = nc.dram_tensor("w2", (4, 128, G4), BF16, kind="ExternalInput")
    if has_bias:
        br1_d = nc.dram_tensor("brow1", (1, G4), BF16, kind="ExternalInput")
        br2_d = nc.dram_tensor("brow2", (1, G4), BF16, kind="ExternalInput")
    sw_d = nc.dram_tensor("sw", (2, 128, VS), BF16, kind="ExternalInput")
    swp_d = nc.dram_tensor("swp", (2, 128, VS, 2), I16, kind="ExternalInput")
    tgi_d = nc.dram_tensor("tgi", (RT, 128, 8), I16, kind="ExternalInput")
    if has_swb:
        swb_d = nc.dram_tensor("swbp", (128, VS), F32, kind="ExternalInput")
    se_d = nc.dram_tensor("se_out", (128, RT * NEXP), F32,
                          kind="ExternalOutput")
    tg_d = nc.dram_tensor("tg_out", (1, BT), F32, kind="ExternalOutput")

    with tile.TileContext(nc) as tc:
        with tc.tile_pool(name="persist", bufs=1) as pp:
            # ---- persistent SBUF ----
            w1_sb = pp.tile([128, 4, G4], BF16, tag="w1")
            w2_sb = pp.tile([128, 4, G4], BF16, tag="w2")
            nc.sync.dma_start(w1_sb[:], w1_d[:].rearrange("k p c -> p k c"))
            nc.sync.dma_start(w2_sb[:], w2_d[:].rearrange("k p c -> p k c"))
            if has_bias:
                br1 = pp.tile([1, G4], BF16, tag="br1")
                br2 = pp.tile([1, G4], BF16, tag="br2")
                nc.sync.dma_start(br1[:], br1_d[:])
                nc.sync.dma_start(br2[:], br2_d[:])
            else:
                br1 = br2 = None
            sw_sb = pp.tile([128, 2, VS], BF16, tag="sw")
            nc.sync.dma_start(sw_sb[:], sw_d[:].rearrange("k p c -> p k c"))
            swp_sb = pp.tile([128, 2, VS, 2], I16, tag="swp")
            nc.sync.dma_start(swp_sb[:],
                              swp_d[:].rearrange("k p c d -> p k c d"))
            if has_swb:
                swb_sb = pp.tile([128, VS], F32, tag="swb")
                nc.sync.dma_start(swb_sb[:], swb_d[:])

            xsT = pp.tile([128, 2, BT], BF16, tag="xsT")
            hsT = pp.tile([128, 2, BT], BF16, tag="hsT")

            ones1 = pp.tile([1, 32], BF16, tag="ones1")
            nc.gpsimd.memset(ones1[:], 1.0)
            onesc = pp.tile([128, 1], BF16, tag="onesc")
            nc.gpsimd.memset(onesc[:], 1.0)
            half_sb = pp.tile([128, 1], F32, tag="half")
            nc.gpsimd.memset(half_sb[:], 0.5)
            ident = pp.tile([128, 128], BF16, tag="ident")
            make_identity(nc, ident[:])

            c1 = pp.tile([128, 64], F32, tag="c1")
            c2 = pp.tile([128, 64], F32, tag="c2")
            nc.gpsimd.memset(c1[:], 0.0)
            nc.gpsimd.memset(c2[:], 0.0)
            junk = pp.tile([128, 1], F32, tag="junk")

            h1T = pp.tile([128, 2, 32], BF16, tag="h1T")

            se_sb = pp.tile([128, RT * NEXP], F32, tag="se")
            tg_sb = pp.tile([1, BT], F32, tag="tg")
            # accum_out adds into existing SBUF content on HW — zero it
            nc.gpsimd.memset(se_sb[:], 0.0)

            with (
                tc.tile_pool(name="stage", bufs=3) as stp,
                tc.tile_pool(name="gwork", bufs=2) as gw,
                tc.tile_pool(name="lwork", bufs=2) as lw,
                tc.tile_pool(name="z1p", bufs=1, space="PSUM") as z1p,
                tc.tile_pool(name="z2p", bufs=1, space="PSUM") as z2p,
                tc.tile_pool(name="lgp", bufs=2, space="PSUM") as lgp,
                tc.tile_pool(name="htp", bufs=1, space="PSUM") as htp,
                tc.tile_pool(name="ptp", bufs=1, space="PSUM") as ptp,
                tc.tile_pool(name="ew", bufs=3) as ew,
            ):
                # ---- embedding gather (time-major) + transpose to slabs ----
                for rt in range(RT):
                    ids_sb = stp.tile([128, 1], I32, tag="ids")
                    nc.gpsimd.dma_start(ids_sb[:], ids_d.ap()[rt])
                    xrow = stp.tile([128, H], BF16, tag="xrow")
                    nc.gpsimd.indirect_dma_start(
                        out=xrow[:], out_offset=None,
                        in_=emb_d[:],
                        in_offset=bass.IndirectOffsetOnAxis(
                            ap=ids_sb[:, :1], axis=0),
                    )
                    cs = 128 * rt
                    nc.sync.dma_start_transpose(
                        xsT[:, 0, cs:cs + 128], xrow[:, 0:128])
                    nc.sync.dma_start_transpose(
                        xsT[:, 1, cs:cs + 128], xrow[:, 128:256])

                def emit_pair(rt, s):
                    """Logits matmuls for chunk-pair s of row-tile rt.
                    Returns a closure that emits the exp (run ~1 step
                    later so the ACT never waits on these matmuls)."""
                    cs = 128 * rt
                    lg = lgp.tile([128, 2, 512], F32, tag="lg")
                    for k in range(2):
                        for half in range(2):
                            ch = s * 2 + half
                            nc.tensor.matmul(
                                lg[:, half, 0:CH],
                                hsT[:, k, cs:cs + 128],
                                sw_sb[:, k, ch * CH:ch * CH + CH],
                                start=(k == 0), stop=(k == 1),
                            )

                    def do_exp():
                        if has_swb:
                            for half in range(2):
                                ch = s * 2 + half
                                nc.vector.tensor_tensor(
                                    out=lg[:, half, 0:CH],
                                    in0=lg[:, half, 0:CH],
                                    in1=swb_sb[:, ch * CH:ch * CH + CH],
                                    op=ALU.add)
                        ebuf = ew.tile([128, 2, CH], BF16, tag="ebuf")
                        col = rt * NEXP + s
                        nc.scalar.activation(
                            ebuf[:], lg[:, :, 0:CH], AF.Exp,
                            accum_out=se_sb[:, col:col + 1])
                    return do_exp

                def emit_tgt(rt):
                    # target logit for row-tile rt's 128 rows
                    cs = 128 * rt
                    tgi_sb = ew.tile([128, 8], I16, tag="tgi")
                    nc.gpsimd.dma_start(tgi_sb[:], tgi_d.ap()[rt])
                    pst = ptp.tile([1, 128], F32, tag="pst")
                    for k in range(2):
                        swg = ew.tile([128, 128, 2], I16, tag="swg")
                        nc.gpsimd.ap_gather(
                            swg[:], swp_sb[:, k], tgi_sb[:],
                            channels=128, num_elems=VS, d=2, num_idxs=128,
                        )
                        mulk = ew.tile([128, 128], BF16, tag="mulk")
                        nc.vector.tensor_tensor(
                            out=mulk[:],
                            in0=swg[:].bitcast(BF16)[:, :, 0],
                            in1=hsT[:, k, cs:cs + 128],
                            op=ALU.mult)
                        nc.tensor.matmul(pst[:], onesc[:, 0:1], mulk[:],
                                         start=(k == 0), stop=(k == 1))
                    nc.scalar.copy(tg_sb[0:1, cs:cs + 128], pst[:])

                def lstm_layer(zpool, w_sb, brow, c_sb, xks, hks, hTdst):
                    """One layer step. xks/hks: list of (lhsT k-tile AP,
                    k index). hTdst: psum AP [128, 64] for the h
                    transposes. Returns h_int."""
                    z = zpool.tile([128, 256], F32, tag="z")
                    nk = len(xks) + len(hks)
                    if has_bias:
                        for j in range(4):
                            nc.tensor.matmul(
                                z[32 * j:32 * j + 32, :], ones1[0:1, :],
                                brow[0:1, 256 * j:256 * j + 256],
                                start=True, stop=False,
                                tile_position=(0, 32 * j),
                                skip_group_check=True)
                    for i, (lt, k) in enumerate(xks + hks):
                        first = (i == 0) and not has_bias
                        last = (i == nk - 1)
                        for j in range(4):
                            nc.tensor.matmul(
                                z[32 * j:32 * j + 32, :], lt,
                                w_sb[:, k, 256 * j:256 * j + 256],
                                start=first, stop=last,
                                tile_position=(0, 32 * j),
                                skip_group_check=True)
                    # gates: cols [f|i|o|j] x 64; forget bias via ACT bias
                    # (tanh((f+1)/2) = tanh(0.5*f_scaled + 0.5))
                    g = gw.tile([128, 256], BF16, tag="g")
                    nc.scalar.activation(g[:, 0:64], z[:, 0:64], AF.Tanh,
                                         bias=half_sb[:, :1])
                    nc.scalar.activation(g[:, 64:256], z[:, 64:256], AF.Tanh)
                    # c = sig(f+1)*c + sig(i)*tanh(j); h = tanh(c)*sig(o)
                    u = lw.tile([128, 64], BF16, tag="u")
                    nc.vector.affine_mul_reduce(
                        u[:], junk[:], g[:, 64:128], g[:, 192:256], 0.5, 0.5)
                    v = lw.tile([128, 64], F32, tag="v")
                    nc.vector.affine_mul_reduce(
                        v[:], junk[:], g[:, 0:64], c_sb[:], 0.5, 0.5)
                    nc.vector.tensor_tensor(out=c_sb[:], in0=u[:], in1=v[:],
                                            op=ALU.add)
                    tc_t = lw.tile([128, 64], BF16, tag="tc")
                    nc.scalar.activation(tc_t[:], c_sb[:], AF.Tanh)
                    hrow = lw.tile([128, 64], BF16, tag="hrow")
                    nc.vector.affine_mul_reduce(
                        hrow[:], junk[:], g[:, 128:192], tc_t[:], 0.5, 0.5)
                    # transpose to hidden-major k-tiles:
                    # quarter j -> [64, 32] at (partition 64*(j%2), col 32*(j//2))
                    for j in range(4):
                        pj = 64 * (j % 2)
                        cj = 32 * (j // 2)
                        nc.tensor.transpose(
                            hTdst[pj:pj + 64, cj:cj + 32],
                            hrow[32 * j:32 * j + 32, :],
                            ident[32 * j:32 * j + 32, 32 * j:32 * j + 32],
                            tile_position=(32 * j, pj),
                        )
                    return hrow

                # ---- LSTM over T steps ----
                exp_q = []
                for t in range(T):
                    ts0 = 32 * t
                    hpair = htp.tile([128, 128], BF16, tag="hpair")
                    # layer 1: x k-tiles (k=0,1) + h1 k-tiles (k=2,3)
                    xks = [(xsT[:, k, ts0:ts0 + 32], k) for k in range(2)]
                    hks = [] if t == 0 else \
                        [(h1T[:, k, :], 2 + k) for k in range(2)]
                    lstm_layer(z1p, w1_sb, br1, c1, xks, hks, hpair[:, 0:64])
                    nc.vector.tensor_copy(h1T[:], hpair[:, 0:64])

                    # layer 2: h2 k-tiles (prev step, k=2,3) + h1 (k=0,1)
                    hks2 = [] if t == 0 else \
                        [(hsT[:, k, ts0 - 32:ts0], 2 + k) for k in range(2)]
                    xks2 = [(h1T[:, k, :], k) for k in range(2)]
                    lstm_layer(z2p, w2_sb, br2, c2, hks2, xks2,
                               hpair[:, 64:128])
                    nc.vector.tensor_copy(hsT[:, 0:2, ts0:ts0 + 32],
                                          hpair[:, 64:128])

                    # softmax: one chunk-pair of tile rt per step, spread
                    # over steps 4rt+3 .. 4rt+6; exp runs one step later
                    if t >= 3:
                        rt, s = divmod(t - 3, 4)
                        if s == 0:
                            emit_tgt(rt)
                        exp_q.append(emit_pair(rt, s))
                    while len(exp_q) > 1:
                        exp_q.pop(0)()

                for tt in range(T, T + 3):
                    rt, s = divmod(tt - 3, 4)
                    if s == 0:
                        emit_tgt(rt)
                    exp_q.append(emit_pair(rt, s))
                while exp_q:
                    exp_q.pop(0)()

            nc.sync.dma_start(se_d[:], se_sb[:])
            nc.sync.dma_start(tg_d[:], tg_sb[:])

    nc.compile()
    meta = dict(T=T, V=V, n_cores=n_cores, B=B, H=H, VS=VS, BT=BT, RT=RT,
                CH=CH, NCHUNK=NCHUNK, NEXP=NEXP)
    return nc, meta


# ---------------- host-side prep / combine ----------------

def prep_inputs(meta, input_data, targets, embedding, W1, b1, W2, b2,
                softmax_w, softmax_b):
    """Build the per-core input maps (numpy)."""
    B, T, V = meta["B"], meta["T"], meta["V"]
    VS, RT, n_cores = meta["VS"], meta["RT"], meta["n_cores"]
    H = meta["H"]
    G4 = 4 * H

    ids_tm = np.ascontiguousarray(
        np.asarray(input_data, np.int64).T).reshape(-1)
    tgt_tm = np.ascontiguousarray(
        np.asarray(targets, np.int64).T).reshape(-1)
    ids_in = ids_tm.astype(np.int32).reshape(RT, 128, 1)

    # W column permutation: new col = 256*jq + 64*g + u  <-  tf col
    # tfblock(g)*256 + 64*jq + u, g order [f,i,o,jnew] -> tf [i,j,f,o]
    tfblock = [2, 0, 3, 1]
    jq = np.arange(G4) // 256
    g = (np.arange(G4) % 256) // 64
    u = np.arange(G4) % 64
    perm = np.array(tfblock)[g] * 256 + 64 * jq + u
    scale = np.where(g < 3, 0.5, 1.0).astype(np.float32)  # f,i,o sigmoid

    def prep_w(W):
        Wp = (np.asarray(W, np.float32)[:, perm] * scale[None, :]).astype(
            ml_dtypes.bfloat16)
        return np.ascontiguousarray(Wp.reshape(4, 128, G4))

    def prep_b(b):
        # input bias only; the forget bias is applied via the ACT bias
        bp = np.asarray(b, np.float32)[perm] * scale
        return np.ascontiguousarray(
            bp.astype(ml_dtypes.bfloat16).reshape(1, G4))

    w1_in = prep_w(W1)
    w2_in = prep_w(W2)
    br1 = prep_b(b1)
    br2 = prep_b(b2)

    emb_in = np.ascontiguousarray(
        np.asarray(embedding, np.float32).astype(ml_dtypes.bfloat16))

    sw = np.asarray(softmax_w, np.float32)                  # [H, V]
    swb = np.asarray(softmax_b, np.float32)
    has_swb = bool(np.any(swb))
    has_bias = bool(np.any(br1) or np.any(br2))

    # vectorized ap_gather index layout: idx i lives at partition i%16,
    # column i//16, replicated per 16-partition group
    rtA = (np.arange(RT) * 128)[:, None, None]
    pA = (np.arange(128) % 16)[None, :, None]
    qA = (np.arange(8) * 16)[None, None, :]
    gat = rtA + qA + pA                                     # [RT, 128, 8]

    maps, masks = [], []
    for c in range(n_cores):
        shard = sw[:, c * VS:(c + 1) * VS].astype(ml_dtypes.bfloat16)
        sw_in = np.ascontiguousarray(shard.reshape(2, 128, VS))
        swi = sw_in.view(np.int16)
        swp_in = np.ascontiguousarray(
            np.stack([swi, swi], axis=-1))                  # [2,128,VS,2]

        tl = tgt_tm - c * VS
        inr = (tl >= 0) & (tl < VS)
        tlc = np.where(inr, tl, 0).astype(np.int16)
        tgi = tlc[gat]                                      # [RT, 128, 8]
        m = dict(ids=ids_in, emb=emb_in, w1=w1_in, w2=w2_in,
                 sw=sw_in, swp=swp_in, tgi=tgi)
        if has_bias:
            m["brow1"] = br1
            m["brow2"] = br2
        if has_swb:
            m["swbp"] = np.ascontiguousarray(
                np.tile(swb[c * VS:(c + 1) * VS].reshape(1, VS), (128, 1)))
        maps.append(m)
        masks.append(inr.astype(np.float32))
    return maps, masks, ids_tm, tgt_tm, has_swb, has_bias


def combine_outputs(meta, results, masks, tgt_tm, softmax_b):
    """results: list of per-core dicts with se_out [128, RT*NEXP] and
    tg_out [1, BT]. Returns the scalar cost (np.float32)."""
    B, T, BT = meta["B"], meta["T"], meta["BT"]
    RT, NEXP = meta["RT"], meta["NEXP"]
    se_all = np.zeros(BT, np.float64)
    tg_all = np.zeros(BT, np.float64)
    for c, r in enumerate(results):
        se = np.asarray(r["se_out"], np.float64)  # [128, RT*NEXP]
        se = se.reshape(128, RT, NEXP).sum(-1)    # [128, RT]
        se_all += se.T.reshape(-1)                # row r = rt*128 + p
        tg_all += np.asarray(r["tg_out"], np.float64)[0] * masks[c]
    tg_all += np.asarray(softmax_b, np.float64)[tgt_tm]
    loss = np.log(se_all) - tg_all
    return np.float32(loss.sum() / B / T)


# ---------------- public entry point ----------------

_CACHE = {}
last_exec_time_ns = None
last_trace_path = None


def _get_built(T_, has_swb, has_bias):
    key = (T_, has_swb, has_bias)
    if key not in _CACHE:
        _CACHE[key] = build_charrnn(T=T_, V=V, n_cores=NCORES,
                                    has_swb=has_swb, has_bias=has_bias,
                                    num_devices=NCORES)
    return _CACHE[key]


def kernel(input_data, targets, embedding, W1, b1, W2, b2,
           softmax_w, softmax_b, _trace=False):
    global last_exec_time_ns, last_trace_path
    T_ = int(np.asarray(input_data).shape[1])
    has_swb = bool(np.any(np.asarray(softmax_b)))
    has_bias = bool(np.any(np.asarray(b1)) or np.any(np.asarray(b2)))
    nc, meta = _get_built(T_, has_swb, has_bias)
    maps, masks, ids_tm, tgt_tm, _, _ = prep_inputs(
        meta, input_data, targets, embedding, W1, b1, W2, b2,
        softmax_w, softmax_b)
    res = run_bass_kernel_spmd(nc, maps, core_ids=list(range(NCORES)),
                               trace=_trace)
    last_exec_time_ns = res.exec_time_ns
    if res.instructions_and_trace is not None:
        last_trace_path = res.instructions_and_trace[1]
    cost = combine_outputs(meta, res.results, masks, tgt_tm, softmax_b)
    return np.asarray(cost, np.float32)


# revision 20
# speedup vs baseline: 1.1400x; 1.0600x over previous
"""Self-contained Trainium2 Bass kernel for the CharRNN problem:
2-layer LSTM (B=32, T=256, H=256) + V=32000 softmax cross-entropy mean loss.

Strategy (8 NeuronCores, SPMD):
  * LSTM recurrence replicated on every core (latency-bound); softmax
    sharded over vocab (VS=4000/core); host combines partial sums.
  * Per-step gates are computed with COLUMN-TILED matmuls
    (tile_position=(0,32j)): partition strip j (rows 32j:32j+32) holds
    batch rows for HIDDEN QUARTER j, with the strip's 256 psum columns
    = [i|o|f|jnew] x 64 units. The 4 strip matmuls stream concurrently
    on the PE sub-arrays, and the gate nonlinearity becomes ONE
    128-partition Tanh ACT instr (sigmoid = 0.5*tanh(x/2)+0.5 with the
    0.5 pre-scaled into W, forget bias injected via a K=1 ones-row
    matmul in the accumulation group).
  * All elementwise state math is [128, 64] (batch x quarter
    interleaved); h is transposed back to hidden-major k-tiles with 4
    tiny PE transposes (tile_position row/col placement) + 1 DVE copy
    instead of 1.2us DMA transposes.
  * Softmax: logits matmuls into 2-bank psum pairs, exp via wide ACT
    instrs with accum_out; per-row target logit via gpsimd ap_gather
    (int16-pair view of sw) + multiply + ones-matmul reduce.
  * Host combines: loss_r = log(sum_cores se_r) - tgt_logit_r.
"""
import os
import numpy as np
import ml_dtypes
import concourse.bass as bass
import concourse.mybir as mybir
import concourse.tile as tile
from concourse import bacc
from concourse.masks import make_identity
from concourse.bass_utils import run_bass_kernel_spmd

F32 = mybir.dt.float32
BF16 = mybir.dt.bfloat16
I32 = mybir.dt.int32
I16 = mybir.dt.int16
AF = mybir.ActivationFunctionType
ALU = mybir.AluOpType

B, T, H, V, NCORES = 32, 256, 256, 32000, 8


def build_charrnn(T=256, V=32000, n_cores=8, has_swb=False, has_bias=False,
                  num_devices=8):
    B, H = 32, 256
    G4 = 4 * H
    VS = V // n_cores
    BT = B * T
    RT = BT // 128                  # 128-row tiles (4 steps each)
    assert T % 4 == 0 and BT % 128 == 0

    CH = 500                        # logits chunk (<=512 = one psum bank)
    NCHUNK = VS // CH               # 8 chunks per tile
    assert VS % CH == 0 and NCHUNK % 2 == 0
    NEXP = NCHUNK // 2              # exp instrs per tile (2 chunks each)

    nc = bacc.Bacc("TRN2", target_bir_lowering=False, debug=False,
                   num_devices=num_devices)

    # ---------------- DRAM I/O ----------------
    ids_d = nc.dram_tensor("ids", (RT, 128, 1), I32, kind="ExternalInput")
    emb_d = nc.dram_tensor("emb", (V, H), BF16, kind="ExternalInput")
    w1_d = nc.dram_tensor("w1", (4, 128, G4), BF16, kind="ExternalInput")
    w2_d = nc.dram_tensor("w2", (4, 128, G4), BF16, kind="ExternalInput")
    br1_d = nc.dram_tensor("brow1", (1, G4), BF16, kind="ExternalInput")
    br2_d = nc.dram_tensor("brow2", (1, G4), BF16, kind="ExternalInput")
    sw_d = nc.dram_tensor("sw", (2, 128, VS), BF16, kind="ExternalInput")
    swp_d = nc.dram_tensor("swp", (2, 128, VS, 2), I16, kind="ExternalInput")
    tgi_d = nc.dram_tensor("tgi", (RT, 128, 8), I16, kind="ExternalInput")
    if has_swb:
        swb_d = nc.dram_tensor("swbp", (128, VS), F32, kind="ExternalInput")
    se_d = nc.dram_tensor("se_out", (128, RT * NEXP), F32,
                          kind="ExternalOutput")
    tg_d = nc.dram_tensor("tg_out", (1, BT), F32, kind="ExternalOutput")

    with tile.TileContext(nc) as tc:
        with tc.tile_pool(name="persist", bufs=1) as pp:
            # ---- persistent SBUF ----
            w1_sb = pp.tile([128, 4, G4], BF16, tag="w1")
            w2_sb = pp.tile([128, 4, G4], BF16, tag="w2")
            nc.sync.dma_start(w1_sb[:], w1_d[:].rearrange("k p c -> p k c"))
            nc.sync.dma_start(w2_sb[:], w2_d[:].rearrange("k p c -> p k c"))
            br1 = pp.tile([1, G4], BF16, tag="br1")
            br2 = pp.tile([1, G4], BF16, tag="br2")
            nc.sync.dma_start(br1[:], br1_d[:])
            nc.sync.dma_start(br2[:], br2_d[:])
            sw_sb = pp.tile([128, 2, VS], BF16, tag="sw")
            nc.sync.dma_start(sw_sb[:], sw_d[:].rearrange("k p c -> p k c"))
            swp_sb = pp.tile([128, 2, VS, 2], I16, tag="swp")
            nc.sync.dma_start(swp_sb[:],
                              swp_d[:].rearrange("k p c d -> p k c d"))
            if has_swb:
                swb_sb = pp.tile([128, VS], F32, tag="swb")
                nc.sync.dma_start(swb_sb[:], swb_d[:])

            xsT = pp.tile([128, 2, BT], BF16, tag="xsT")
            hsT = pp.tile([128, 2, BT], BF16, tag="hsT")

            ones1 = pp.tile([1, 32], BF16, tag="ones1")
            nc.gpsimd.memset(ones1[:], 1.0)
            onesc = pp.tile([128, 1], BF16, tag="onesc")
            nc.gpsimd.memset(onesc[:], 1.0)
            half_sb = pp.tile([128, 1], F32, tag="half")
            nc.gpsimd.memset(half_sb[:], 0.5)
            ident = pp.tile([128, 128], BF16, tag="ident")
            make_identity(nc, ident[:])

            c1 = pp.tile([128, 64], F32, tag="c1")
            c2 = pp.tile([128, 64], F32, tag="c2")
            nc.gpsimd.memset(c1[:], 0.0)
            nc.gpsimd.memset(c2[:], 0.0)
            junk = pp.tile([128, 1], F32, tag="junk")

            h1T = pp.tile([128, 2, 32], BF16, tag="h1T")

            se_sb = pp.tile([128, RT * NEXP], F32, tag="se")
            tg_sb = pp.tile([1, BT], F32, tag="tg")
            # accum_out adds into existing SBUF content on HW — zero it
            nc.gpsimd.memset(se_sb[:], 0.0)

            with (
                tc.tile_pool(name="stage", bufs=3) as stp,
                tc.tile_pool(name="gwork", bufs=2) as gw,
                tc.tile_pool(name="lwork", bufs=2) as lw,
                tc.tile_pool(name="z1p", bufs=1, space="PSUM") as z1p,
                tc.tile_pool(name="z2p", bufs=1, space="PSUM") as z2p,
                tc.tile_pool(name="lgp", bufs=2, space="PSUM") as lgp,
                tc.tile_pool(name="htp", bufs=1, space="PSUM") as htp,
                tc.tile_pool(name="ptp", bufs=1, space="PSUM") as ptp,
                tc.tile_pool(name="ew", bufs=3) as ew,
            ):
                # ---- embedding gather (time-major) + transpose to slabs ----
                for rt in range(RT):
                    ids_sb = stp.tile([128, 1], I32, tag="ids")
                    nc.gpsimd.dma_start(ids_sb[:], ids_d.ap()[rt])
                    xrow = stp.tile([128, H], BF16, tag="xrow")
                    nc.gpsimd.indirect_dma_start(
                        out=xrow[:], out_offset=None,
                        in_=emb_d[:],
                        in_offset=bass.IndirectOffsetOnAxis(
                            ap=ids_sb[:, :1], axis=0),
                    )
                    cs = 128 * rt
                    nc.sync.dma_start_transpose(
                        xsT[:, 0, cs:cs + 128], xrow[:, 0:128])
                    nc.sync.dma_start_transpose(
                        xsT[:, 1, cs:cs + 128], xrow[:, 128:256])

                def emit_pair(rt, s):
                    """Logits matmuls for chunk-pair s of row-tile rt.
                    Returns a closure that emits the exp (run ~1 step
                    later so the ACT never waits on these matmuls)."""
                    cs = 128 * rt
                    lg = lgp.tile([128, 2, 512], F32, tag="lg")
                    for k in range(2):
                        for half in range(2):
                            ch = s * 2 + half
                            nc.tensor.matmul(
                                lg[:, half, 0:CH],
                                hsT[:, k, cs:cs + 128],
                                sw_sb[:, k, ch * CH:ch * CH + CH],
                                start=(k == 0), stop=(k == 1),
                            )

                    def do_exp():
                        if has_swb:
                            for half in range(2):
                                ch = s * 2 + half
                                nc.vector.tensor_tensor(
                                    out=lg[:, half, 0:CH],
                                    in0=lg[:, half, 0:CH],
                                    in1=swb_sb[:, ch * CH:ch * CH + CH],
                                    op=ALU.add)
                        ebuf = ew.tile([128, 2, CH], BF16, tag="ebuf")
                        col = rt * NEXP + s
                        nc.scalar.activation(
                            ebuf[:], lg[:, :, 0:CH], AF.Exp,
                            accum_out=se_sb[:, col:col + 1])
                    return do_exp

                def emit_tgt(rt):
                    # target logit for row-tile rt's 128 rows
                    cs = 128 * rt
                    tgi_sb = ew.tile([128, 8], I16, tag="tgi")
                    nc.gpsimd.dma_start(tgi_sb[:], tgi_d.ap()[rt])
                    pst = ptp.tile([1, 128], F32, tag="pst")
                    for k in range(2):
                        swg = ew.tile([128, 128, 2], I16, tag="swg")
                        nc.gpsimd.ap_gather(
                            swg[:], swp_sb[:, k], tgi_sb[:],
                            channels=128, num_elems=VS, d=2, num_idxs=128,
                        )
                        mulk = ew.tile([128, 128], BF16, tag="mulk")
                        nc.vector.tensor_tensor(
                            out=mulk[:],
                            in0=swg[:].bitcast(BF16)[:, :, 0],
                            in1=hsT[:, k, cs:cs + 128],
                            op=ALU.mult)
                        nc.tensor.matmul(pst[:], onesc[:, 0:1], mulk[:],
                                         start=(k == 0), stop=(k == 1))
                    nc.scalar.copy(tg_sb[0:1, cs:cs + 128], pst[:])

                def lstm_layer(zpool, w_sb, brow, c_sb, xks, hks, hTdst):
                    """One layer step. xks/hks: list of (lhsT k-tile AP,
                    k index). hTdst: psum AP [128, 64] for the h
                    transposes. Returns h_int."""
                    z = zpool.tile([128, 256], F32, tag="z")
                    nk = len(xks) + len(hks)
                    for j in range(4):
                        nc.tensor.matmul(
                            z[32 * j:32 * j + 32, :], ones1[0:1, :],
                            brow[0:1, 256 * j:256 * j + 256],
                            start=True, stop=False,
                            tile_position=(0, 32 * j),
                            skip_group_check=True)
                    for i, (lt, k) in enumerate(xks + hks):
                        last = (i == nk - 1)
                        for j in range(4):
                            nc.tensor.matmul(
                                z[32 * j:32 * j + 32, :], lt,
                                w_sb[:, k, 256 * j:256 * j + 256],
                                start=False, stop=last,
                                tile_position=(0, 32 * j),
                                skip_group_check=True)
                    # one Tanh for all gates: cols [f|i|o|j] x 64
                    # (forget bias rides in the brow bias matmuls)
                    g = gw.tile([128, 256], BF16, tag="g")
                    nc.scalar.activation(g[:], z[:], AF.Tanh)
                    # c = sig(f+1)*c + sig(i)*tanh(j); h = tanh(c)*sig(o)
                    u = lw.tile([128, 64], BF16, tag="u")
                    nc.vector.affine_mul_reduce(
                        u[:], junk[:], g[:, 64:128], g[:, 192:256], 0.5, 0.5)
                    v = lw.tile([128, 64], F32, tag="v")
                    nc.vector.affine_mul_reduce(
                        v[:], junk[:], g[:, 0:64], c_sb[:], 0.5, 0.5)
                    nc.vector.tensor_tensor(out=c_sb[:], in0=u[:], in1=v[:],
                                            op=ALU.add)
                    tc_t = lw.tile([128, 64], BF16, tag="tc")
                    nc.scalar.activation(tc_t[:], c_sb[:], AF.Tanh)
                    hrow = lw.tile([128, 64], BF16, tag="hrow")
                    nc.vector.affine_mul_reduce(
                        hrow[:], junk[:], g[:, 128:192], tc_t[:], 0.5, 0.5)
                    # transpose to hidden-major k-tiles:
                    # quarter j -> [64, 32] at (partition 64*(j%2), col 32*(j//2))
                    for j in range(4):
                        pj = 64 * (j % 2)
                        cj = 32 * (j // 2)
                        nc.tensor.transpose(
                            hTdst[pj:pj + 64, cj:cj + 32],
                            hrow[32 * j:32 * j + 32, :],
                            ident[32 * j:32 * j + 32, 32 * j:32 * j + 32],
                            tile_position=(32 * j, pj),
                        )
                    return hrow

                # ---- LSTM over T steps ----
                exp_q = []
                for t in range(T):
                    ts0 = 32 * t
                    hpair = htp.tile([128, 128], BF16, tag="hpair")
                    # layer 1: x k-tiles (k=0,1) + h1 k-tiles (k=2,3)
                    xks = [(xsT[:, k, ts0:ts0 + 32], k) for k in range(2)]
                    hks = [] if t == 0 else \
                        [(h1T[:, k, :], 2 + k) for k in range(2)]
                    lstm_layer(z1p, w1_sb, br1, c1, xks, hks, hpair[:, 0:64])
                    nc.vector.tensor_copy(h1T[:], hpair[:, 0:64])

                    # layer 2: h2 k-tiles (prev step, k=2,3) + h1 (k=0,1)
                    hks2 = [] if t == 0 else \
                        [(hsT[:, k, ts0 - 32:ts0], 2 + k) for k in range(2)]
                    xks2 = [(h1T[:, k, :], k) for k in range(2)]
                    lstm_layer(z2p, w2_sb, br2, c2, hks2, xks2,
                               hpair[:, 64:128])
                    nc.vector.tensor_copy(hsT[:, 0:2, ts0:ts0 + 32],
                                          hpair[:, 64:128])

                    # softmax: one chunk-pair of tile rt per step, spread
                    # over steps 4rt+3 .. 4rt+6; exp runs one step later
                    if t >= 3:
                        rt, s = divmod(t - 3, 4)
                        if s == 0:
                            emit_tgt(rt)
                        exp_q.append(emit_pair(rt, s))
                    while len(exp_q) > 1:
                        exp_q.pop(0)()

                for tt in range(T, T + 3):
                    rt, s = divmod(tt - 3, 4)
                    if s == 0:
                        emit_tgt(rt)
                    exp_q.append(emit_pair(rt, s))
                while exp_q:
                    exp_q.pop(0)()

            nc.sync.dma_start(se_d[:], se_sb[:])
            nc.sync.dma_start(tg_d[:], tg_sb[:])

    nc.compile()
    meta = dict(T=T, V=V, n_cores=n_cores, B=B, H=H, VS=VS, BT=BT, RT=RT,
                CH=CH, NCHUNK=NCHUNK, NEXP=NEXP)
    return nc, meta


# ---------------- host-side prep / combine ----------------

def prep_inputs(meta, input_data, targets, embedding, W1, b1, W2, b2,
                softmax_w, softmax_b):
    """Build the per-core input maps (numpy)."""
    B, T, V = meta["B"], meta["T"], meta["V"]
    VS, RT, n_cores = meta["VS"], meta["RT"], meta["n_cores"]
    H = meta["H"]
    G4 = 4 * H

    ids_tm = np.ascontiguousarray(
        np.asarray(input_data, np.int64).T).reshape(-1)
    tgt_tm = np.ascontiguousarray(
        np.asarray(targets, np.int64).T).reshape(-1)
    ids_in = ids_tm.astype(np.int32).reshape(RT, 128, 1)

    # W column permutation: new col = 256*jq + 64*g + u  <-  tf col
    # tfblock(g)*256 + 64*jq + u, g order [f,i,o,jnew] -> tf [i,j,f,o]
    tfblock = [2, 0, 3, 1]
    jq = np.arange(G4) // 256
    g = (np.arange(G4) % 256) // 64
    u = np.arange(G4) % 64
    perm = np.array(tfblock)[g] * 256 + 64 * jq + u
    scale = np.where(g < 3, 0.5, 1.0).astype(np.float32)  # f,i,o sigmoid

    def prep_w(W):
        Wp = (np.asarray(W, np.float32)[:, perm] * scale[None, :]).astype(
            ml_dtypes.bfloat16)
        return np.ascontiguousarray(Wp.reshape(4, 128, G4))

    def prep_b(b):
        bp = np.asarray(b, np.float32)[perm] * scale
        bp = bp + np.where(g == 0, 0.5, 0.0)      # forget bias (scaled)
        return np.ascontiguousarray(
            bp.astype(ml_dtypes.bfloat16).reshape(1, G4))

    w1_in = prep_w(W1)
    w2_in = prep_w(W2)
    br1 = prep_b(b1)
    br2 = prep_b(b2)

    emb_in = np.ascontiguousarray(
        np.asarray(embedding, np.float32).astype(ml_dtypes.bfloat16))

    sw = np.asarray(softmax_w, np.float32)                  # [H, V]
    swb = np.asarray(softmax_b, np.float32)
    has_swb = bool(np.any(swb))
    has_bias = bool(np.any(br1) or np.any(br2))

    # vectorized ap_gather index layout: idx i lives at partition i%16,
    # column i//16, replicated per 16-partition group
    rtA = (np.arange(RT) * 128)[:, None, None]
    pA = (np.arange(128) % 16)[None, :, None]
    qA = (np.arange(8) * 16)[None, None, :]
    gat = rtA + qA + pA                                     # [RT, 128, 8]

    maps, masks = [], []
    for c in range(n_cores):
        shard = sw[:, c * VS:(c + 1) * VS].astype(ml_dtypes.bfloat16)
        sw_in = np.ascontiguousarray(shard.reshape(2, 128, VS))
        swi = sw_in.view(np.int16)
        swp_in = np.ascontiguousarray(
            np.stack([swi, swi], axis=-1))                  # [2,128,VS,2]

        tl = tgt_tm - c * VS
        inr = (tl >= 0) & (tl < VS)
        tlc = np.where(inr, tl, 0).astype(np.int16)
        tgi = tlc[gat]                                      # [RT, 128, 8]
        m = dict(ids=ids_in, emb=emb_in, w1=w1_in, w2=w2_in,
                 brow1=br1, brow2=br2, sw=sw_in, swp=swp_in, tgi=tgi)
        if has_swb:
            m["swbp"] = np.ascontiguousarray(
                np.tile(swb[c * VS:(c + 1) * VS].reshape(1, VS), (128, 1)))
        maps.append(m)
        masks.append(inr.astype(np.float32))
    return maps, masks, ids_tm, tgt_tm, has_swb, has_bias


def combine_outputs(meta, results, masks, tgt_tm, softmax_b):
    """results: list of per-core dicts with se_out [128, RT*NEXP] and
    tg_out [1, BT]. Returns the scalar cost (np.float32)."""
    B, T, BT = meta["B"], meta["T"], meta["BT"]
    RT, NEXP = meta["RT"], meta["NEXP"]
    se_all = np.zeros(BT, np.float64)
    tg_all = np.zeros(BT, np.float64)
    for c, r in enumerate(results):
        se = np.asarray(r["se_out"], np.float64)  # [128, RT*NEXP]
        se = se.reshape(128, RT, NEXP).sum(-1)    # [128, RT]
        se_all += se.T.reshape(-1)                # row r = rt*128 + p
        tg_all += np.asarray(r["tg_out"], np.float64)[0] * masks[c]
    tg_all += np.asarray(softmax_b, np.float64)[tgt_tm]
    loss = np.log(se_all) - tg_all
    return np.float32(loss.sum() / B / T)


# ---------------- public entry point ----------------

_CACHE = {}
last_exec_time_ns = None
last_trace_path = None


def _get_built(T_, has_swb, has_bias):
    key = (T_, has_swb, has_bias)
    if key not in _CACHE:
        _CACHE[key] = build_charrnn(T=T_, V=V, n_cores=NCORES,
                                    has_swb=has_swb, has_bias=has_bias,
                                    num_devices=NCORES)
    return _CACHE[key]


def kernel(input_data, targets, embedding, W1, b1, W2, b2,
           softmax_w, softmax_b, _trace=False):
    global last_exec_time_ns, last_trace_path
    T_ = int(np.asarray(input_data).shape[1])
    has_swb = bool(np.any(np.asarray(softmax_b)))
    has_bias = bool(np.any(np.asarray(b1)) or np.any(np.asarray(b2)))
    nc, meta = _get_built(T_, has_swb, has_bias)
    maps, masks, ids_tm, tgt_tm, _, _ = prep_inputs(
        meta, input_data, targets, embedding, W1, b1, W2, b2,
        softmax_w, softmax_b)
    res = run_bass_kernel_spmd(nc, maps, core_ids=list(range(NCORES)),
                               trace=_trace)
    last_exec_time_ns = res.exec_time_ns
    if res.instructions_and_trace is not None:
        last_trace_path = res.instructions_and_trace[1]
    cost = combine_outputs(meta, res.results, masks, tgt_tm, softmax_b)
    return np.asarray(cost, np.float32)
